# revision 14
# baseline (speedup 1.0000x reference)
"""GCN2 (16-layer) kernel for 8 Trainium2 NeuronCores (self-contained).

Sharding: nodes split 8 ways by destination (12800 padded rows/core).
Per layer:
  - SpMM: dma_gather of source rows from a replicated bf16 feature table
    (4 slab tensors, int16 indices), rotated across 4 SWDGE queues so the
    DMA drains overlap; PE matmuls against host-built one-hot weight
    matrices S (segment sums accumulate in PSUM; slab-major schedule with
    an SBUF accumulator). A host-side balancer permutes nodes so nearly
    every (slab, tile) segment fits in 2 chunks (minimal gather padding).
  - Dense phase in transposed [feature, node] layout: initial-residual mix
    (alpha/beta constants folded into S and immediates), a@W via PE,
    BatchNorm stats via DVE reduce + AllReduce, apply+residual+relu,
    transpose back, per-slab AllGather of the new table (ping-pong
    buffers) so next-layer gathers start after the first slab lands.
"""
import numpy as np
import ml_dtypes

import jax
import concourse.bacc as bacc
import concourse.mybir as mybir
import concourse.tile as tile
from concourse.masks import make_identity

# problem constants (hardcoded per spec)
N = 100000
E = 800000
D = 128
D_OUT = 40
L = 16
ALPHA = 0.1
THETA = 0.5
BN_EPS = 1e-5

NC = 8
P = 128
SH = N // NC            # 12500 real rows per core
NSLAB = 4
QSLAB = 3200            # padded per-core per-slab quota
QREAL = SH // NSLAB     # 3125 real nodes per (core, slab)
SHP = NSLAB * QSLAB     # 12800 padded rows per core
NT = SHP // P           # 100 tiles
TPS = NT // NSLAB       # 25 tiles per slab
SLABSZ = NC * QSLAB     # 25600 table rows per slab (int16-safe)
CHUNK = 128             # edge slots per matmul chunk (contraction K)
MAXCH = 40              # max chunks per dma_gather call / S batch
NQ = 4                  # SWDGE queues for gather rotation

BETAS = [float(np.log(THETA / l + 1.0)) for l in range(1, L + 1)]
AT = mybir.AluOpType

_cache = {}


def _balance(edge_row, edge_col):
    """Assign each node a slab + within-core position so that per
    (core, src-slab, dest-tile) edge counts stay below 2*CHUNK."""
    rng = np.random.default_rng(12345)
    slab = np.empty(N, dtype=np.int64)
    for c in range(NC):
        p = rng.permutation(SH)
        for s in range(NSLAB):
            slab[c * SH + p[s * QREAL:(s + 1) * QREAL]] = s

    # per-dest in-degree vector split by source slab
    vec = np.zeros((N, NSLAB), dtype=np.int64)
    np.add.at(vec, (edge_row, slab[edge_col]), 1)

    perm = np.empty((NC, SH), dtype=np.int64)
    for c in range(NC):
        gbase = c * SH
        sl = slab[gbase:gbase + SH]
        for sd in range(NSLAB):
            raws = np.flatnonzero(sl == sd)          # raw local ids
            v = vec[gbase + raws]                    # [QREAL, NSLAB]
            order = np.argsort(-(v.max(1) * 1000 + v.sum(1)),
                               kind="stable")
            loads = np.zeros((TPS, NSLAB), dtype=np.int64)
            counts = np.zeros(TPS, dtype=np.int64)
            tl_of = np.empty(len(raws), dtype=np.int64)
            m_of = np.empty(len(raws), dtype=np.int64)
            for i in order:
                vi = v[i]
                cand = (loads + vi).max(axis=1) * 256 + counts
                cand[counts >= P] = 1 << 40
                t = int(np.argmin(cand))
                tl_of[i] = t
                m_of[i] = counts[t]
                counts[t] += 1
                loads[t] += vi
            perm[c, raws] = sd * QSLAB + tl_of * P + m_of
    return slab, perm


def _prep(edge_row, edge_col, edge_weight):
    """Host preprocessing: balancer, per-core gather indices, S matrices,
    schedule."""
    edge_row = np.asarray(edge_row).astype(np.int64)
    edge_col = np.asarray(edge_col).astype(np.int64)
    w = np.asarray(edge_weight).astype(np.float64)

    slab, perm = _balance(edge_row, edge_col)

    c_dest = edge_row // SH
    j_dest = perm[c_dest, edge_row % SH]
    t_arr = j_dest // P
    m_arr = j_dest % P

    c_src = edge_col // SH
    j_src = perm[c_src, edge_col % SH]
    s_arr = j_src // QSLAB
    psrc = j_src % QSLAB
    idx_arr = c_src * QSLAB + (psrc % P) * TPS + (psrc // P)

    counts = np.zeros((NC, NSLAB, NT), dtype=np.int64)
    np.add.at(counts, (c_dest, s_arr, t_arr), 1)
    K_st = np.ceil(counts / CHUNK).astype(np.int64).max(axis=0)  # [NSLAB, NT]
    K_st[0] = np.maximum(K_st[0], 1)

    seg_start = np.zeros((NSLAB, NT), dtype=np.int64)
    c = 0
    for s in range(NSLAB):
        for t in range(NT):
            seg_start[s, t] = c
            c += K_st[s, t]
    C_total = c
    total_slots = C_total * CHUNK

    # tile groups (multiples of 4 tiles so 512-col stat blocks align);
    # per group+slab: one gather call of at most MAXCH chunks
    groups = []
    t0g = 0
    while t0g < NT:
        nt = 4
        while (t0g + nt + 4 <= NT
               and K_st[:, t0g:t0g + nt + 4].sum(axis=1).max() <= MAXCH):
            nt += 4
        groups.append((t0g, nt))
        t0g += nt

    # calls ordered group-major, slab-minor: group g's tiles finalize
    # right after its slab-3 call, overlapping later groups' gathers
    callmap = np.zeros(total_slots, dtype=np.int64)
    call_meta = []  # (slab, chunk0, nchunks, idx_col0, queue, t0g, ntg, last)
    colbase = 0
    qi = 0
    for (t0g, ntg) in groups:
        for s in range(NSLAB):
            c0 = int(seg_start[s, t0g])
            nch = int(K_st[s, t0g:t0g + ntg].sum())
            if nch == 0 and s < NSLAB - 1:
                continue
            nidx = nch * CHUNK
            g = np.arange(nidx)
            callmap[c0 * CHUNK + g] = (colbase + g // 16) * 16 + (g % 16)
            call_meta.append((s, c0, nch, colbase, qi % NQ, t0g, ntg,
                              s == NSLAB - 1))
            qi += 1
            colbase += nidx // 16
    idx_cols = colbase

    idx16 = np.zeros((NC, 16, idx_cols), dtype=np.int16)
    S = np.zeros((NC, P, C_total, P), dtype=ml_dtypes.bfloat16)
    order = np.lexsort((idx_arr, t_arr, s_arr, c_dest))
    eo_core = c_dest[order]
    eo_s = s_arr[order]
    eo_t = t_arr[order]
    eo_m = m_arr[order]
    eo_idx = idx_arr[order]
    eo_w = (w[order] * (1.0 - ALPHA)).astype(np.float32)

    for ci in range(NC):
        msk = eo_core == ci
        es, et = eo_s[msk], eo_t[msk]
        seg_id = es * NT + et
        n = len(seg_id)
        pos = np.zeros(n, dtype=np.int64)
        if n:
            change = np.concatenate([[True], seg_id[1:] != seg_id[:-1]])
            starts = np.flatnonzero(change)
            run = np.arange(n)
            pos = run - np.repeat(run[starts],
                                  np.diff(np.concatenate([starts, [n]])))
        slot = seg_start[es, et] * CHUNK + pos
        gc = callmap[slot]
        idx16[ci, gc % 16, gc // 16] = eo_idx[msk].astype(np.int16)
        S[ci, slot % CHUNK, slot // CHUNK, eo_m[msk]] = eo_w[msk]
    idx16 = np.tile(idx16, (1, 8, 1))

    sched = {
        "C_total": int(C_total),
        "idx_cols": int(idx_cols),
        "call_meta": call_meta,
        "seg_start": seg_start,
        "K_st": K_st,
        "perm": perm,
    }
    return sched, idx16, S


def _build_nc(sched, n_layers=L, parts=15):
    nc = bacc.Bacc("TRN2", num_swdge_queues=NQ)
    bf16 = mybir.dt.bfloat16
    f32 = mybir.dt.float32
    C_total = sched["C_total"]
    idx_cols = sched["idx_cols"]
    call_meta = sched["call_meta"]
    seg_start = sched["seg_start"]
    K_st = sched["K_st"]

    xT_in = nc.dram_tensor("xT", [P, SHP], f32, kind="ExternalInput")
    idx_in = nc.dram_tensor("idx16", [P, idx_cols], mybir.dt.int16, kind="ExternalInput")
    S_in = nc.dram_tensor("S", [P, C_total, P], bf16, kind="ExternalInput")
    Win_in = nc.dram_tensor("W_in", [P, P], f32, kind="ExternalInput")
    binT_in = nc.dram_tensor("b_inT", [P, 1], f32, kind="ExternalInput")
    Wst_in = nc.dram_tensor("W_stat", [P, L * P], bf16, kind="ExternalInput")
    gamT_in = nc.dram_tensor("gammaT", [P, L], f32, kind="ExternalInput")
    betT_in = nc.dram_tensor("betaT", [P, L], f32, kind="ExternalInput")
    Wout_in = nc.dram_tensor("W_out", [P, D_OUT], f32, kind="ExternalInput")
    bout_in = nc.dram_tensor("b_outR", [P, D_OUT], f32, kind="ExternalInput")
    out_ext = nc.dram_tensor("out", [P, NT, D_OUT], f32, kind="ExternalOutput")

    tabs = [[nc.dram_tensor(f"tab{i}_{s}", [SLABSZ, P], bf16,
                            addr_space="Shared") for s in range(NSLAB)]
            for i in range(2)]
    ag_ins = [[nc.dram_tensor(f"ag_in{i}_{s}", [QSLAB, P], bf16)
               for s in range(NSLAB)] for i in range(2)]
    ar_ins = [nc.dram_tensor(f"ar_in{i}", [P, 2], f32) for i in range(2)]
    ar_outs = [nc.dram_tensor(f"ar_out{i}", [P, 2], f32, addr_space="Shared")
               for i in range(2)]
    rg = [list(range(NC))]

    with tile.TileContext(nc) as tc:
        with tc.tile_pool(name="persist", bufs=1) as pp, \
             tc.tile_pool(name="msgs", bufs=6) as mp, \
             tc.tile_pool(name="spool", bufs=2) as sp, \
             tc.tile_pool(name="sc", bufs=2) as scp, \
             tc.tile_pool(name="ps_spmm", bufs=3, space="PSUM") as ps_spmm, \
             tc.tile_pool(name="ps_tr", bufs=2, space="PSUM") as ps_tr, \
             tc.tile_pool(name="ps_w", bufs=2, space="PSUM") as ps_w:

            iden = pp.tile([P, P], bf16)
            make_identity(nc, iden[:])
            idnf = pp.tile([P, P], f32)
            make_identity(nc, idnf[:])
            idxt = pp.tile([P, idx_cols], mybir.dt.int16)
            nc.sync.dma_start(idxt[:], idx_in[:])
            # x0A = ALPHA * x0 (ALPHA folded into W_in/b_in host-side)
            x0A = pp.tile([P, SHP], bf16)
            hT = pp.tile([P, SHP], bf16)
            # accum doubles as the AllGather transpose staging buffer
            accum = pp.tile([P, NT, P], bf16)
            mT = pp.tile([P, SHP], bf16)
            WinT = pp.tile([P, P], f32)
            binT = pp.tile([P, 1], f32)
            Wst = pp.tile([P, L * P], bf16)
            gamT = pp.tile([P, L], f32)
            betT = pp.tile([P, L], f32)
            WoutT = pp.tile([P, D_OUT], f32)
            boutT = pp.tile([P, D_OUT], f32)
            stats = pp.tile([P, 2], f32)
            NB = SHP // 512
            sump = pp.tile([P, NB], f32)
            sqp = pp.tile([P, NB], f32)
            bnv = pp.tile([P, 6], f32)
            nc.sync.dma_start(WinT[:], Win_in[:])
            nc.sync.dma_start(binT[:], binT_in[:])
            nc.sync.dma_start(Wst[:], Wst_in[:])
            nc.sync.dma_start(gamT[:], gamT_in[:])
            nc.sync.dma_start(betT[:], betT_in[:])
            nc.sync.dma_start(WoutT[:], Wout_in[:])
            nc.sync.dma_start(boutT[:], bout_in[:])

            # first PE op: depends only on gpsimd-made identity
            wps = ps_tr.tile([P, P], bf16, space="PSUM", tag="tr")
            nc.tensor.transpose(wps[:], iden[:], iden[:])
            # absorb the W_in load on PE
            waps = ps_tr.tile([P, P], f32, space="PSUM", tag="trf", bufs=1)
            nc.tensor.transpose(waps[:2, :], WinT[:, 0:2], idnf[:])

            # ---- x0 stage: x0A = relu((x @ a*W_in)^T + a*b_in) = ALPHA*x0 ----
            for t in range(NT):
                xt = scp.tile([P, P], f32, tag="xtile")
                nc.sync.dma_start(xt[:], xT_in[:, t * P:(t + 1) * P])
                pa = ps_tr.tile([P, P], f32, space="PSUM", tag="trf", bufs=1)
                nc.tensor.transpose(pa[:2, :], xt[:, 0:2], idnf[:])
                ps = ps_w.tile([P, 512], f32, space="PSUM", tag="w")
                nc.tensor.matmul(ps[:, :P], lhsT=WinT[:], rhs=xt[:],
                                 start=True, stop=True)
                nc.vector.tensor_scalar(
                    out=x0A[:, t * P:(t + 1) * P], in0=ps[:, :P],
                    scalar1=binT[:, :1], scalar2=0.0,
                    op0=AT.add, op1=AT.max)
            nc.vector.tensor_scalar_mul(hT[:], x0A[:], float(1.0 / ALPHA))

            def emit_table_update(li):
                pg = li % 2
                for s in range(NSLAB):
                    for tl in range(TPS):
                        t = s * TPS + tl
                        pst = ps_tr.tile([P, P], bf16, space="PSUM", tag="tr")
                        nc.tensor.transpose(pst[:], hT[:, t * P:(t + 1) * P],
                                            iden[:])
                        nc.vector.tensor_copy(accum[:, t, :], pst[:])
                    nc.sync.dma_start(
                        ag_ins[pg][s].ap().rearrange("(m t) f -> m t f", t=TPS),
                        accum[:, s * TPS:(s + 1) * TPS, :])
                    nc.gpsimd.collective_compute(
                        "AllGather", AT.bypass,
                        ins=[ag_ins[pg][s].ap().opt()],
                        outs=[tabs[pg][s].ap().opt()],
                        replica_groups=rg)

            emit_table_update(0)

            seg_of = {}
            for s in range(NSLAB):
                for t in range(NT):
                    c0 = int(seg_start[s, t])
                    k = int(K_st[s, t])
                    for j in range(k):
                        seg_of[c0 + j] = (s, t, j == 0, j == k - 1)

            for li in range(n_layers):
                beta = BETAS[li]
                mc = float(1.0 - beta)
                pg = li % 2

                # --- SpMM, group-pipelined: after a group's slab-3 call,
                #     immediately mix + a@W + stats for its tiles while
                #     later groups' gathers drain ---
                WL = Wst[:, li * P:(li + 1) * P]
                psums = {}
                for (s, c0_call, nch, col0, q, t0g, ntg, last) in call_meta:
                    if nch > 0:
                        st = sp.tile([P, MAXCH, P], bf16, tag="S")
                        nc.scalar.dma_start(st[:, :nch, :],
                                            S_in[:, c0_call:c0_call + nch, :])
                        msgs = mp.tile([P, MAXCH, P], bf16, tag="msgs")
                        nidx = nch * CHUNK
                        nc.gpsimd.dma_gather(
                            msgs[:, :nch, :], tabs[pg][s][:, :],
                            idxt[:, col0:col0 + nidx // 16],
                            nidx, nidx, P, single_packet=False, queue_num=q)
                        pa = ps_tr.tile([P, P], bf16, space="PSUM", tag="tr")
                        nc.tensor.transpose(pa[:2, :], msgs[:, 0, 0:2], iden[:])
                        pa2 = ps_tr.tile([P, P], bf16, space="PSUM", tag="tr")
                        nc.tensor.transpose(pa2[:2, :], st[:, 0, 0:2], iden[:])
                        for j in range(nch):
                            ch = c0_call + j
                            ss, tt, segfirst, seglast = seg_of[ch]
                            if segfirst:
                                psums[tt] = ps_spmm.tile(
                                    [P, P], f32, space="PSUM", tag="spmm",
                                    name="pspmm")
                            nc.tensor.matmul(psums[tt][:], lhsT=st[:, j, :],
                                             rhs=msgs[:, j, :],
                                             start=segfirst, stop=seglast)
                            if seglast:
                                if ss == 0:
                                    nc.vector.tensor_copy(accum[:, tt, :],
                                                          psums[tt][:])
                                else:
                                    nc.vector.tensor_tensor(
                                        out=accum[:, tt, :],
                                        in0=accum[:, tt, :],
                                        in1=psums[tt][:], op=AT.add)
                                del psums[tt]
                    if not last or parts < 2:
                        continue
                    # mix (z-space): mT = accum^T + x0A for this group
                    for t in range(t0g, t0g + ntg):
                        pst = ps_tr.tile([P, P], bf16, space="PSUM", tag="tr")
                        nc.tensor.transpose(pst[:], accum[:, t, :], iden[:])
                        sl = slice(t * P, (t + 1) * P)
                        nc.vector.tensor_tensor(out=mT[:, sl], in0=pst[:],
                                                in1=x0A[:, sl], op=AT.add)
                    # z += (z @ W_stat)^T with fused per-block stats;
                    # W_stat = beta/(1-beta)*W, the (1-beta) scale is folded
                    # into the BN constants below
                    for t0 in range(t0g * P, (t0g + ntg) * P, 512):
                        bi = t0 // 512
                        n = min(512, SHP - t0)
                        psw = ps_w.tile([P, 512], f32, space="PSUM", tag="w")
                        nc.tensor.matmul(psw[:, :n], lhsT=WL,
                                         rhs=mT[:, t0:t0 + n],
                                         start=True, stop=True)
                        nc.vector.scalar_tensor_tensor(
                            out=mT[:, t0:t0 + n], in0=mT[:, t0:t0 + n],
                            scalar=0.0, in1=psw[:, :n], op0=AT.add, op1=AT.add,
                            accum_out=sump[:, bi:bi + 1])
                        scr = scp.tile([P, 512], bf16, tag="sqscr")
                        nc.vector.scalar_tensor_tensor(
                            out=scr[:, :n], in0=mT[:, t0:t0 + n], scalar=0.0,
                            in1=mT[:, t0:t0 + n], op0=AT.add, op1=AT.mult,
                            accum_out=sqp[:, bi:bi + 1])

                # --- BN stats + AllReduce ---
                if parts < 4:
                    continue
                nc.vector.tensor_reduce(out=stats[:, 0:1], in_=sump[:],
                                        axis=mybir.AxisListType.X, op=AT.add)
                nc.vector.tensor_reduce(out=stats[:, 1:2], in_=sqp[:],
                                        axis=mybir.AxisListType.X, op=AT.add)
                nc.sync.dma_start(ar_ins[pg][:], stats[:])
                nc.gpsimd.collective_compute(
                    "AllReduce", AT.add,
                    ins=[ar_ins[pg].ap().opt()], outs=[ar_outs[pg].ap().opt()],
                    replica_groups=rg)
                arr = scp.tile([P, 2], f32, tag="arres")
                nc.sync.dma_start(arr[:], ar_outs[pg][:])
                if parts < 8:
                    continue
                # a = mc*z; mean_a = mc*sum_z/N; var_a = mc^2*(msq_z - mean_z^2)
                nc.vector.tensor_scalar_mul(bnv[:, 0:1], arr[:, 0:1],
                                            float(mc / N))
                nc.vector.tensor_scalar_mul(bnv[:, 1:2], arr[:, 1:2],
                                            float(mc * mc / N))
                nc.vector.tensor_tensor(out=bnv[:, 2:3], in0=bnv[:, 0:1],
                                        in1=bnv[:, 0:1], op=AT.mult)
                nc.vector.tensor_tensor(out=bnv[:, 2:3], in0=bnv[:, 1:2],
                                        in1=bnv[:, 2:3], op=AT.subtract)
                nc.vector.tensor_scalar_add(bnv[:, 2:3], bnv[:, 2:3], BN_EPS)
                nc.scalar.sqrt(bnv[:, 3:4], bnv[:, 2:3])
                nc.vector.reciprocal(bnv[:, 3:4], bnv[:, 3:4])
                # scale' = mc * gamma * rsqrt(var+eps)  (applied to z)
                nc.vector.tensor_tensor(out=bnv[:, 4:5], in0=bnv[:, 3:4],
                                        in1=gamT[:, li:li + 1], op=AT.mult)
                nc.vector.tensor_scalar_mul(bnv[:, 4:5], bnv[:, 4:5], mc)
                nc.vector.tensor_tensor(out=bnv[:, 5:6], in0=bnv[:, 0:1],
                                        in1=bnv[:, 3:4], op=AT.mult)
                nc.vector.tensor_tensor(out=bnv[:, 5:6], in0=bnv[:, 5:6],
                                        in1=gamT[:, li:li + 1], op=AT.mult)
                nc.vector.tensor_tensor(out=bnv[:, 5:6],
                                        in0=betT[:, li:li + 1],
                                        in1=bnv[:, 5:6], op=AT.subtract)

                # --- apply + residual + relu + table update, per slab so the
                #     AllGather for slab 0 fires early ---
                pg2 = (li + 1) % 2
                for s in range(NSLAB):
                    sl = slice(s * QSLAB, (s + 1) * QSLAB)
                    nc.vector.tensor_scalar(out=mT[:, sl], in0=mT[:, sl],
                                            scalar1=bnv[:, 4:5],
                                            scalar2=bnv[:, 5:6],
                                            op0=AT.mult, op1=AT.add)
                    nc.vector.tensor_tensor(out=hT[:, sl], in0=mT[:, sl],
                                            in1=hT[:, sl], op=AT.add)
                    nc.vector.tensor_scalar_max(hT[:, sl], hT[:, sl], 0.0)
                    if li < n_layers - 1:
                        for tl in range(TPS):
                            t = s * TPS + tl
                            pst = ps_tr.tile([P, P], bf16, space="PSUM",
                                             tag="tr")
                            nc.tensor.transpose(pst[:],
                                                hT[:, t * P:(t + 1) * P],
                                                iden[:])
                            nc.vector.tensor_copy(accum[:, t, :], pst[:])
                        nc.sync.dma_start(
                            ag_ins[pg2][s].ap().rearrange(
                                "(m t) f -> m t f", t=TPS),
                            accum[:, s * TPS:(s + 1) * TPS, :])
                        nc.gpsimd.collective_compute(
                            "AllGather", AT.bypass,
                            ins=[ag_ins[pg2][s].ap().opt()],
                            outs=[tabs[pg2][s].ap().opt()],
                            replica_groups=rg)

            # ---- output ----
            for t in range(NT):
                hsl = scp.tile([P, P], f32, tag="hf32")
                nc.vector.tensor_copy(hsl[:], hT[:, t * P:(t + 1) * P])
                pso = ps_w.tile([P, 512], f32, space="PSUM", tag="w")
                nc.tensor.matmul(pso[:, :D_OUT], lhsT=hsl[:], rhs=WoutT[:],
                                 start=True, stop=True)
                osl = scp.tile([P, D_OUT], f32, tag="otile")
                nc.vector.tensor_tensor(out=osl[:], in0=pso[:, :D_OUT],
                                        in1=boutT[:], op=AT.add)
                nc.sync.dma_start(out_ext[:, t, :], osl[:])
    nc.compile()
    return nc


def _make_inputs(inputs):
    W_in = np.asarray(inputs["W_in"], dtype=np.float32)
    b_in = np.asarray(inputs["b_in"], dtype=np.float32)
    conv_W = np.asarray(inputs["conv_W"], dtype=np.float32)
    bn_gamma = np.asarray(inputs["bn_gamma"], dtype=np.float32)
    bn_beta = np.asarray(inputs["bn_beta"], dtype=np.float32)
    W_out = np.asarray(inputs["W_out"], dtype=np.float32)
    b_out = np.asarray(inputs["b_out"], dtype=np.float32)

    W_stat = np.stack([conv_W[l] * (BETAS[l] / (1.0 - BETAS[l]))
                       for l in range(L)])  # [L, P, P]
    W_stat = np.ascontiguousarray(
        W_stat.transpose(1, 0, 2).reshape(P, L * P)).astype(ml_dtypes.bfloat16)
    shared = {
        "W_in": np.ascontiguousarray(W_in * ALPHA),
        "b_inT": np.ascontiguousarray(b_in[:, None] * ALPHA),
        "W_stat": W_stat,
        "gammaT": np.ascontiguousarray(bn_gamma.T),
        "betaT": np.ascontiguousarray(bn_beta.T),
        "W_out": W_out,
        "b_outR": np.ascontiguousarray(np.tile(b_out[None, :], (P, 1))),
    }
    return shared


def _stage(inputs, perm, idx16, S):
    shared = _make_inputs(inputs)
    x = np.asarray(inputs["x"], dtype=np.float32)
    in_maps = []
    for c in range(NC):
        xs = np.zeros((SHP, P), dtype=np.float32)
        xs[perm[c]] = x[c * SH:(c + 1) * SH]
        m = dict(shared)
        m["xT"] = np.ascontiguousarray(xs.T)
        m["idx16"] = idx16[c]
        m["S"] = np.ascontiguousarray(S[c])
        in_maps.append(m)
    return in_maps


def kernel(**inputs):
    if "runner" not in _cache:
        sched, idx16, S = _prep(inputs["edge_row"], inputs["edge_col"],
                                inputs["edge_weight"])
        nc = _build_nc(sched)
        r = _SpmdRunner(nc, NC)
        _cache["runner"] = (r, sched, idx16, S)
    r, sched, idx16, S = _cache["runner"]
    perm = sched["perm"]
    in_maps = _stage(inputs, perm, idx16, S)
    dev_in = r.stage_inputs(in_maps)
    outs = r.run(dev_in)
    res = r.results(outs)
    full = np.zeros((N, D_OUT), dtype=np.float32)
    for c in range(NC):
        o = res[c]["out"].transpose(1, 0, 2).reshape(SHP, D_OUT)
        full[c * SH:(c + 1) * SH] = o[perm[c]]
    return full


class _SpmdRunner:
    """Jit-once SPMD execution of a Bass module via PJRT/axon."""

    def __init__(self, nc, n_cores):
        from jax.sharding import Mesh, PartitionSpec
        from jax.experimental.shard_map import shard_map
        from concourse.bass2jax import (_bass_exec_p, install_neuronx_cc_hook,
                                        partition_id_tensor)
        install_neuronx_cc_hook()
        self.nc = nc
        self.n_cores = n_cores
        self.PartitionSpec = PartitionSpec
        self.shard_map = shard_map

        in_names, out_names, out_avals, zero_outs = [], [], [], []
        pname = nc.partition_id_tensor.name if nc.partition_id_tensor else None
        for alloc in nc.m.functions[0].allocations:
            if not isinstance(alloc, mybir.MemoryLocationSet):
                continue
            name = alloc.memorylocations[0].name
            if alloc.kind == "ExternalInput":
                if name != pname:
                    in_names.append(name)
            elif alloc.kind == "ExternalOutput":
                shape = tuple(alloc.tensor_shape)
                dtype = mybir.dt.np(alloc.dtype)
                out_names.append(name)
                out_avals.append(jax.core.ShapedArray(shape, dtype))
                zero_outs.append(np.zeros(shape, dtype))
        self.in_names, self.out_names = in_names, out_names
        self.out_avals, self.zero_outs = out_avals, zero_outs
        n_params, n_outs = len(in_names), len(out_names)
        self.n_params = n_params
        all_in = list(in_names) + list(out_names)
        if pname is not None:
            all_in.append(pname)
        donate = tuple(range(n_params, n_params + n_outs))

        def _body(*args):
            operands = list(args)
            if pname is not None:
                operands.append(partition_id_tensor())
            return tuple(_bass_exec_p.bind(
                *operands, out_avals=tuple(out_avals),
                in_names=tuple(all_in), out_names=tuple(out_names),
                lowering_input_output_aliases=(),
                sim_require_finite=True, sim_require_nnan=True, nc=nc))

        devices = jax.devices()[:n_cores]
        self.mesh = Mesh(np.asarray(devices), ("core",))
        self.fn = jax.jit(
            shard_map(_body, mesh=self.mesh,
                      in_specs=(PartitionSpec("core"),) * (n_params + n_outs),
                      out_specs=(PartitionSpec("core"),) * n_outs,
                      check_rep=False),
            donate_argnums=donate, keep_unused=True)

    def _ident(self, n):
        key = ("ident", n)
        if not hasattr(self, "_idents"):
            self._idents = {}
        if key not in self._idents:
            PS = self.PartitionSpec
            self._idents[key] = jax.jit(self.shard_map(
                lambda *a: tuple(a), mesh=self.mesh,
                in_specs=(PS("core"),) * n, out_specs=(PS("core"),) * n,
                check_rep=False))
        return self._idents[key]

    def stage_inputs(self, in_maps):
        per_core = [[np.asarray(m[n]) for n in self.in_names] for m in in_maps]
        concat = [np.concatenate([per_core[c][i] for c in range(self.n_cores)],
                                 axis=0) for i in range(self.n_params)]
        out = self._ident(len(concat))(*concat)
        jax.block_until_ready(out)
        return list(out)

    def _zero_args(self):
        zeros = [np.zeros((self.n_cores * z.shape[0], *z.shape[1:]), z.dtype)
                 for z in self.zero_outs]
        if not zeros:
            return []
        out = self._ident(len(zeros))(*zeros)
        jax.block_until_ready(out)
        return list(out)

    def run(self, dev_in):
        outs = self.fn(*dev_in, *self._zero_args())
        jax.block_until_ready(outs)
        return outs

    def results(self, outs):
        return [{name: np.asarray(outs[i]).reshape(
                    self.n_cores, *self.out_avals[i].shape)[c]
                 for i, name in enumerate(self.out_names)}
                for c in range(self.n_cores)]

    def time_runs(self, dev_in, iters=5):
        import time
        ts = []
        for _ in range(iters):
            za = self._zero_args()
            t0 = time.perf_counter()
            outs = self.fn(*dev_in, *za)
            jax.block_until_ready(outs)
            ts.append(time.perf_counter() - t0)
        return min(ts), ts
